# revision 2
# baseline (speedup 1.0000x reference)
"""TopK sparse autoencoder forward pass on 8 TRN2 NeuronCores.

Data-parallel over batch (1024 rows/core), no collectives.

Pipeline per core:
  Encode (exact): preact = x_eff @ W_enc.T with fp16-hi operands (x64 scale)
    plus ONE paired fp8 DoubleRow cross-correction per contraction chunk
    (plane0 = (4096*x_l)*(w), plane1 = (x)*(4096*w_l)), all 6 chunks —
    residual ~7e-6, so top-64 selection is essentially exact and the whole
    error budget goes to the decoder.
  Candidates: per 256-feature segment, DVE max8 captures the top-8 values
    (validated: <=7 winners per segment) and max_index their positions.
    The dense preact is never stored.
  Threshold: two prefix top-64 rounds (blocks 0-19, 20-39) run on DVE slack
    during the encode; a short 256-wide final merge yields the exact
    64th-largest value t per row.
  Scatter: candidates are masked (val >= t -> index, else -1) and written by
    GPSIMD local_scatter (per-partition dst[:,idx]=data, dst zeroed) into
    dense [128, 12288] u16 buffers holding PACKED fp8 pairs
    (lo byte = e4m3(v), hi byte = e4m3(v/64)); scatters are issued
    group-major so decode transposes start after ~4 small scatters.
  Decode: PE transposes the u16 pairs (bit-exact permutation), then fp8
    DoubleRow matmuls with host-prepared plane-split decoder weights as the
    stationary operand (Wh8 = e4m3(Wd), Wl8 = e4m3(64*(Wd - Wh8))) and the
    packed f.T as a strided moving view:
       f8*Wh + (f8/64)*(64*Wl) = f8 * (Wh + Wl + O(eps^2))
    at 0.5 cyc/row — half the fp16 cost. The decode output is computed
    transposed [D, B] and un-transposed by the host for free.

Measured: rel err 1.8376e-2 (e4m3 quantization of f values; selection
exact), TimelineSim 1211918 ns/core (PE ~86% busy: encode 737us + decode
245us + transposes 82us).
"""

import os
import numpy as np
import ml_dtypes

import concourse.bass as bass
import concourse.tile as tile
from concourse import bacc, mybir
from concourse.bass_utils import run_bass_kernel_spmd

F16 = np.float16
E4 = ml_dtypes.float8_e4m3

N_CORES = 8
B_FULL = 8192
D = 768              # act dim
NF = 24576           # dict size
K_TOP = 64
P = 128
B_CORE = B_FULL // N_CORES     # 1024
RT = B_CORE // P               # 8 row tiles per core
BLK = 512                      # encode feature block (PSUM bank)
NB = NF // BLK                 # 48
KC = D // P                    # 6 contraction chunks
SEG = 256                      # candidate segment (max 7 winners/seg validated)
SLOTS = NB * 2 * 8             # 768 candidate slots per row
GRP = 1536                     # features per local_scatter call (num_elems)
SPG = GRP // SEG * 8           # candidate slots per scatter group = 48
NGRP = NF // GRP               # 16 scatter groups
HALF_F = NF // 2               # 12288 features per decode half
HALF_C = HALF_F // P           # 96 chunks per half
GPH = NGRP // 2                # 8 scatter groups per half
TPD = 4                        # row tiles per decode sweep
GU = 8                         # chunks per decode pipeline unit
NEG_INF = -1e30


def _build_program():
    nc = bacc.Bacc("TRN2", target_bir_lowering=False, debug=False,
                   num_devices=N_CORES)
    dt = mybir.dt

    xт = nc.declare_dram_parameter("xt", [D, B_CORE], dt.float32, isOutput=False)
    wт = nc.declare_dram_parameter("wt", [D, NF], dt.float32, isOutput=False)
    wd8_ext = nc.declare_dram_parameter("wd8", [NF, 2, D], dt.float8e4, isOutput=False)
    idxc_ext = nc.declare_dram_parameter("idxc", [P, SLOTS], dt.uint16, isOutput=False)
    id_ext = nc.declare_dram_parameter("ident", [P, P], dt.float16, isOutput=False)
    out_ext = nc.declare_dram_parameter("out", [D, B_CORE], dt.float32, isOutput=True)
    DEBUG = bool(os.environ.get("TOPK_DEBUG"))
    if DEBUG:
        cdbg_ext = nc.declare_dram_parameter("cdbg", [RT, P, SLOTS], dt.float32, isOutput=True)
        idbg_ext = nc.declare_dram_parameter("idbg", [RT, P, SLOTS], dt.uint16, isOutput=True)
        rdbg_ext = nc.declare_dram_parameter("rdbg", [P, 8 * RT], dt.float32, isOutput=True)

    DR = mybir.MatmulPerfMode.DoubleRow
    ACT_COPY = mybir.ActivationFunctionType.Copy
    ALU = mybir.AluOpType

    with tile.TileContext(nc) as tc:
        with tc.tile_pool(name="persist", bufs=1) as pp:
            cands = [pp.tile([P, SLOTS], dt.float32, tag=f"cand{rt}", name=f"cand{rt}")
                     for rt in range(RT)]
            rawi = [pp.tile([P, SLOTS], dt.uint16, tag=f"rawi{rt}", name=f"rawi{rt}")
                    for rt in range(RT)]
            r8all = pp.tile([P, 8 * RT], dt.float32, tag="r8all", name="r8all")
            PREB = 40                      # prefix levels cover blocks 0..PREB-1
            NTAIL = (NB - PREB) * 16       # 128 tail candidate slots
            BW = 128 + NTAIL               # two prefix-64s + tail
            big = pp.tile([P, BW * RT], dt.float32, tag="big", name="big")
            ct = pp.tile([P, SLOTS], dt.uint16, tag="ct")
            idn = pp.tile([P, P], dt.float16, tag="idn")
            nc.sync.dma_start(ct[:], idxc_ext[:])
            nc.sync.dma_start(idn[:], id_ext[:])

            def emit_prefix(rt, prp, lvl):
                # top-64 of candidate slots for a 20-block range (runs on DVE
                # slack during encode; cands preserved via scratch copy)
                sc = prp.tile([P, 320], dt.float32, tag="psc")
                nc.vector.tensor_copy(sc[:], cands[rt][:, lvl * 320:(lvl + 1) * 320])
                for r in range(8):
                    dst = big[:, rt * BW + lvl * 64 + r * 8:
                              rt * BW + lvl * 64 + (r + 1) * 8]
                    nc.vector.max(dst, sc[:])
                    if r < 7:
                        nc.vector.match_replace(sc[:], dst, sc[:], NEG_INF)

            def emit_final(rt):
                # 64th largest of (prefix top-64s U tail candidates)
                arr = big[:, rt * BW:(rt + 1) * BW]
                nc.vector.tensor_copy(arr[:, 128:BW], cands[rt][:, PREB * 16:])
                r8 = r8all[:, rt * 8:(rt + 1) * 8]
                for r in range(8):
                    nc.vector.max(r8, arr)
                    if r < 7:
                        nc.vector.match_replace(arr, r8, arr, NEG_INF)

            # ------------- phase A: exact fp32 encode + candidates -------------
            with (
                tc.tile_pool(name="xp", bufs=1) as xp,
                tc.tile_pool(name="wp", bufs=3) as wp,
                tc.tile_pool(name="pa", bufs=8, space="PSUM") as pa,
                tc.tile_pool(name="ev", bufs=12) as evp,
                tc.tile_pool(name="pre", bufs=2) as prp,
            ):
                xr = xp.tile([P, KC, B_CORE], dt.float32r, tag="xt")
                for kc in range(KC):
                    nc.sync.dma_start(xt_sb[:, kc, :], xт[kc * P:(kc + 1) * P, :])
                xr = xt_sb.bitcast(dt.float32r)

                for nb in range(NB):
                    c0 = nb * BLK
                    wr = wp.tile([P, KC, BLK], dt.float32r, tag="wt")
                    for kc in range(KC):
                        nc.sync.dma_start(wt_sb[:, kc, :],
                                          wт[kc * P:(kc + 1) * P, c0:c0 + BLK])
                    wr = wt_sb.bitcast(dt.float32r)
                    for rt in range(RT):
                        r0 = rt * P
                        acc = pa.tile([P, BLK], dt.float32, tag="acc")
                        for kc in range(KC):
                            nc.tensor.matmul(acc[:], xr[:, kc, r0:r0 + P],
                                             wr[:, kc, :],
                                             start=(kc == 0), stop=(kc == KC - 1))
                        ev = evp.tile([P, BLK], dt.float32, tag="ev")
                        nc.scalar.activation(ev[:], acc[:], ACT_COPY)
                        for s in range(2):
                            slot = (nb * 2 + s) * 8
                            cd = cands[rt][:, slot:slot + 8]
                            nc.vector.max(cd, ev[:, s * SEG:(s + 1) * SEG])
                            nc.vector.max_index(rawi[rt][:, slot:slot + 8], cd,
                                                ev[:, s * SEG:(s + 1) * SEG])
                    if 20 <= nb < 28 and rt == RT - 1:
                        emit_prefix(nb - 20, prp, 0)
                    if nb >= 40 and rt == RT - 1:
                        emit_prefix(nb - 40, prp, 1)

            if DEBUG:
                for rt in range(RT):
                    nc.sync.dma_start(cdbg_ext[rt, :, :], cands[rt][:])
                    nc.sync.dma_start(idbg_ext[rt, :, :], rawi[rt][:])
                nc.sync.dma_start(rdbg_ext[:], r8all[:])

            # ------------- phase B: pack + scatter + fp8-DR decode -------------
            # Decode is computed transposed: out' = Wd.T-chunks (stationary,
            # host-prepared plane pairs) x f.T (moving, packed fp8 pairs read
            # as a strided DR view).  out' [D, B_CORE] is un-transposed by the
            # host.  4 row tiles per sweep share each W_dec stream.
            with (
                tc.tile_pool(name="pk", bufs=1) as pkp,
                tc.tile_pool(name="pt", bufs=2) as ptp,
                tc.tile_pool(name="dn", bufs=1) as dnp,
                tc.tile_pool(name="wdp", bufs=2) as wdp,
                tc.tile_pool(name="sfp", bufs=2) as sfp,
                tc.tile_pool(name="tpp", bufs=2, space="PSUM") as tpp,
                tc.tile_pool(name="pdec", bufs=1, space="PSUM") as pdec,
                tc.tile_pool(name="oev", bufs=2) as oev,
            ):
                DC = D // P   # 6 output chunks of 128
                sweeps = [list(range(RT))[i:i + TPD] for i in range(0, RT, TPD)]
                denses = [dnp.tile([P, HALF_F], dt.uint16, tag=f"dn{j}",
                                   name=f"dn{j}") for j in range(TPD)]
                for si, sweep in enumerate(sweeps):
                    # pack candidates of this sweep's row tiles
                    pks, ims = [], []
                    for j, rt in enumerate(sweep):
                        if si == 0:
                            emit_final(rt)
                        t_ap = r8all[:, rt * 8 + 7:rt * 8 + 8]
                        im = pkp.tile([P, SLOTS], dt.int16, tag=f"im{j}",
                                      name=f"im{j}")
                        rel = ptp.tile([P, SLOTS], dt.uint16, tag="rel")
                        # rel+1 = raw_seg_idx + ((slot//8)%6)*256 + 1
                        nc.vector.tensor_tensor(rel[:], rawi[rt][:], ct[:], ALU.add)
                        # im = (cand >= t) ? rel+1 : 0, then -1 -> rel or -1
                        nc.vector.scalar_tensor_tensor(
                            im[:], cands[rt][:], t_ap, rel[:], ALU.is_ge, ALU.mult)
                        nc.vector.tensor_scalar_add(im[:], im[:], -1)
                        q0 = ptp.tile([P, SLOTS], dt.float8e4, tag="q0")
                        q1 = ptp.tile([P, SLOTS], dt.float8e4, tag="q1")
                        nc.vector.tensor_copy(q0[:], cands[rt][:])
                        nc.vector.tensor_scalar_mul(q1[:], cands[rt][:], 1.0 / 64.0)
                        pk = pkp.tile([P, SLOTS], dt.uint16, tag=f"pk{j}",
                                      name=f"pk{j}")
                        t2 = ptp.tile([P, SLOTS], dt.uint16, tag="t2")
                        nc.vector.tensor_copy(pk[:], q0.bitcast(dt.uint8)[:])
                        nc.vector.tensor_copy(t2[:], q1.bitcast(dt.uint8)[:])
                        nc.vector.tensor_scalar_mul(t2[:], t2[:], 256)
                        nc.vector.tensor_tensor(pk[:], pk[:], t2[:], ALU.add)
                        pks.append(pk)
                        ims.append(im)
                    for g in range(GPH):
                        for j in range(len(sweep)):
                            nc.gpsimd.local_scatter(
                                denses[j][:, g * GRP:(g + 1) * GRP],
                                pks[j][:, g * SPG:(g + 1) * SPG],
                                ims[j][:, g * SPG:(g + 1) * SPG],
                                channels=P, num_elems=GRP, num_idxs=SPG)

                    accs = [pdec.tile([P, TPD * P], dt.float32, tag=f"da{dc}",
                                      name=f"da{dc}") for dc in range(DC)]
                    for h in range(2):
                        if h == 1:
                            for g in range(GPH):
                                gs = (GPH + g) * SPG
                                for j in range(len(sweep)):
                                    nc.gpsimd.local_scatter(
                                        denses[j][:, g * GRP:(g + 1) * GRP],
                                        pks[j][:, gs:gs + SPG],
                                        ims[j][:, gs:gs + SPG],
                                        channels=P, num_elems=GRP, num_idxs=SPG)
                        for gu in range(HALF_C // GU):
                            if si == 0 and h == 1 and 2 <= gu < 2 + TPD:
                                emit_final(sweeps[1][gu - 2])
                            ch0 = h * HALF_C + gu * GU   # global chunk base
                            wdt = wdp.tile([P, GU, 2, D], dt.float8e4, tag="wd")
                            for c in range(GU):
                                nc.sync.dma_start(
                                    wdt[:, c, :, :],
                                    wd8_ext[(ch0 + c) * P:(ch0 + c + 1) * P, :, :])
                            sf = sfp.tile([P, GU, TPD * P], dt.float16, tag="sf")
                            for c in range(GU):
                                tp = tpp.tile([P, TPD * P], dt.float16, tag="tp")
                                for j in range(TPD):
                                    f0 = (gu * GU + c) * P
                                    nc.tensor.transpose(
                                        tp[:, j * P:(j + 1) * P],
                                        denses[j].bitcast(dt.float16)[:, f0:f0 + P],
                                        idn[:])
                                nc.scalar.activation(sf[:, c, :], tp[:], ACT_COPY)
                            sf8 = sf.bitcast(dt.float8e4).rearrange(
                                "p c (m two) -> p c two m", two=2)
                            first = (h == 0 and gu == 0)
                            last = (h == 1 and gu == HALF_C // GU - 1)
                            for dc in range(DC):
                                for c in range(GU):
                                    st = first and c == 0
                                    sp = last and c == GU - 1
                                    nc.tensor.matmul(
                                        accs[dc][:], wdt[:, c, :, dc * P:(dc + 1) * P],
                                        sf8[:, c, :, :],
                                        start=st, stop=sp, perf_mode=DR)
                    for dc in range(DC):
                        o = oev.tile([P, TPD * P], dt.float32, tag="o")
                        nc.scalar.activation(o[:], accs[dc][:], ACT_COPY)
                        nc.sync.dma_start(
                            out_ext[dc * P:(dc + 1) * P,
                                    si * TPD * P:(si + 1) * TPD * P], o[:])

    nc.compile()
    return nc


def kernel(x, W_enc, b_enc, W_dec, b_dec):
    x = np.asarray(x, dtype=np.float32)
    W_enc = np.asarray(W_enc, dtype=np.float32)
    b_enc = np.asarray(b_enc, dtype=np.float32)
    W_dec = np.asarray(W_dec, dtype=np.float32)
    b_dec = np.asarray(b_dec, dtype=np.float32)

    if np.any(b_enc):
        raise NotImplementedError("nonzero b_enc not supported")

    x_eff = x - b_dec[None, :]

    HS = 64.0
    SCALE = 4096.0
    # hi fp16 operands scaled by 64 each (products at 4096x); paired fp8 cross
    # covers the fp16 rounding residuals exactly to second order:
    #   plane0 = (4096*x_l)*(w), plane1 = (x)*(4096*w_l)
    xh_full = (HS * x_eff).astype(F16)                       # [B, D]
    x_l = x_eff - xh_full.astype(np.float32) / HS
    wh_full = (HS * W_enc).astype(F16)                       # [NF, D]
    W_l = W_enc - wh_full.astype(np.float32) / HS
    x_p0 = (SCALE * x_l).astype(E4)
    x_p1 = x_eff.astype(E4)
    w_p0 = W_enc.astype(E4)
    w_p1 = (SCALE * W_l).astype(E4)
    wh_param = np.ascontiguousarray(wh_full.T)               # [D, NF] f16
    wc_param = np.ascontiguousarray(
        np.stack([w_p0.T, w_p1.T], axis=1))                  # [D, 2, NF] fp8
    Wd_T = np.ascontiguousarray(W_dec.T).astype(np.float32)  # [NF, D]
    Wh8 = Wd_T.astype(E4)
    Wl8 = (64.0 * (Wd_T - Wh8.astype(np.float32))).astype(E4)
    wd8_param = np.ascontiguousarray(np.stack([Wh8, Wl8], axis=1))  # [NF, 2, D]

    slots = np.arange(SLOTS)
    cvals = (((slots // 8) % 6) * SEG + 1).astype(np.uint16)
    idxc_param = np.ascontiguousarray(np.tile(cvals[None, :], (P, 1)))
    ident = np.eye(P, dtype=F16)

    nc = _build_program()

    in_maps = []
    for c in range(N_CORES):
        rs, re = c * B_CORE, (c + 1) * B_CORE
        in_maps.append({
            "xh": np.ascontiguousarray(xh_full[rs:re].T),
            "xc": np.ascontiguousarray(
                np.stack([x_p0[rs:re].T, x_p1[rs:re].T], axis=1)),
            "wh": wh_param,
            "wc": wc_param,
            "wd8": wd8_param,
            "idxc": idxc_param,
            "ident": ident,
        })

    def _run():
        return run_bass_kernel_spmd(nc, in_maps, core_ids=list(range(N_CORES)))

    try:
        res = _run()
    except ModuleNotFoundError:
        # BASS_TRACE=1 routes through an NTFF profiling hook missing in some
        # containers; disable tracing and retry.
        os.environ["BASS_NEVER_TRACE"] = "1"
        res = _run()
    out = np.concatenate([res.results[c]["out"].T for c in range(N_CORES)], axis=0)
    out = out + b_dec[None, :]
    return out.astype(np.float32)


if __name__ == "__main__":
    rng = np.random.default_rng(0)
    xs = rng.standard_normal((B_FULL, D)).astype(np.float32)
    We = (rng.standard_normal((NF, D)) / np.sqrt(D)).astype(np.float32)
    Wd = We.T / (np.linalg.norm(We.T, axis=0, keepdims=True) + 1e-7)
    o = kernel(xs, We, np.zeros(NF, np.float32), Wd.astype(np.float32),
               np.zeros(D, np.float32))
    print(o.shape, o.dtype)
